# revision 1
# baseline (speedup 1.0000x reference)
"""TRN2 Bass kernel for nn_CAM_Module (channel attention over packed point-cloud scenes).

Math per segment (n rows, C=256 channels), with X = segment viewed as [C, n]
(a pure reshape of the row-major [n, C] buffer):
    G    = X @ X.T                      # [C, C] Gram over the flat axis
    attn = softmax(rowmax(G) - G)       # == exp(rowmin(G) - G) / rowsum (shift cancels)
    out  = gamma * (attn @ X) + X       # viewed back as [n, C]

Sharding: 8 segments -> 8 NeuronCores, fully local per core.

Implementation per core:
  Phase 1: PE-transpose f32 X tiles ([k,c] layout), split hi/lo bf16 on the far
           side (ACT cast + DVE sub from PSUM), G = Xh@[Xh|Xl].T in one packed
           [128,512] matmul per c-half per k-subtile; Ghl^T term added by
           symmetry. (lo*lo dropped: ~1e-3 error on entries of scale 65536.)
  Phase 2: softmax + fold gamma and the residual identity into B = gamma*attn^T + I.
  Phase 3: out = B.T @ X in float32r (full-rate PE at N>=512, ~12-bit mantissa,
           rounding done for free by SWDGE cast-DMA loads); PSUM drained by
           ACT/DVE alternately.
"""

import numpy as np

BATCHES = 8
C = 256
N_SEG = 65536  # rows per segment

_nc_cache = {}


def _build(n_seg: int, debug=False):
    """Emit the Bass program for one core (one segment of n_seg rows)."""
    from contextlib import ExitStack

    import concourse.bass as bass
    import concourse.tile as tile
    from concourse import bacc, mybir
    from concourse.masks import make_identity

    f32 = mybir.dt.float32
    f32r = mybir.dt.float32r
    bf16 = mybir.dt.bfloat16

    # x flat has n_seg*C elements; X = [C, n_seg] view.
    KLEN = n_seg
    KT = 4096  # k-tile for phase 1
    JT = 4096  # j-tile for phase 3
    assert KLEN % KT == 0 and KLEN % JT == 0

    nc = bacc.Bacc("TRN2", target_bir_lowering=False, debug=False, num_devices=8)

    x = nc.dram_tensor("x", [n_seg, C], f32, kind="ExternalInput").ap()
    gamma = nc.dram_tensor("gamma", [1], f32, kind="ExternalInput").ap()
    out = nc.dram_tensor("out", [n_seg, C], f32, kind="ExternalOutput").ap()
    dbg = None
    if debug:
        dbg = {
            "g_dbg": nc.dram_tensor("g_dbg", [C, C], f32, kind="ExternalOutput").ap(),
            "b_dbg": nc.dram_tensor("b_dbg", [C, C], f32, kind="ExternalOutput").ap(),
        }

    # [C, KLEN] views of the flat buffer (pure reshape, row-major)
    xv = x.rearrange("(c r) ch -> c (r ch)", c=C)
    ov = out.rearrange("(c r) ch -> c (r ch)", c=C)

    with tile.TileContext(nc) as tc, ExitStack() as ctx:
        const = ctx.enter_context(tc.tile_pool(name="const", bufs=1))

        ident_f32 = const.tile([128, 128], f32)
        make_identity(nc, ident_f32[:])

        # I_dh[p, c] = 1.0 iff c == p + 128*dh   (residual identity, [d, c] layout)
        eye = []
        for dh in range(2):
            t = const.tile([128, C], f32, tag=f"eye{dh}", name=f"eye{dh}")
            nc.gpsimd.memset(t[:], 0.0)
            nc.gpsimd.affine_select(
                out=t[:],
                in_=t[:],
                compare_op=mybir.AluOpType.not_equal,
                fill=1.0,
                base=128 * dh,
                pattern=[[-1, C]],
                channel_multiplier=1,
            )
            eye.append(t)

        g_sb = const.tile([128, 1], f32)
        g_bcast = bass.AP(tensor=gamma.tensor, offset=gamma.offset, ap=[[0, 128], [1, 1]])
        nc.gpsimd.dma_start(out=g_sb[:], in_=g_bcast)

        # B tiles (gamma*attn^T + I), f32r, [d-half, c-full]; filled in phase 2
        b_t = [const.tile([128, C], f32r, tag=f"bt{dh}", name=f"bt{dh}") for dh in range(2)]

        # SBUF caches of X (f32r) so phase 3 skips/preloads those DMA reads:
        # head j-tile filled by cast-DMA issued NOW (runs in phase 1's idle DMA,
        # bridges the phase boundary); tail k-tiles Pool-cast from phase 1's xf.
        NCACHE_KT = 2 if n_seg == 65536 else 0
        NHEAD = 1 if n_seg == 65536 else 0
        cache = ctx.enter_context(tc.tile_pool(name="xcache", bufs=1))
        cache_t = {}
        for cjt in range(NHEAD):
            for dh in range(2):
                t = cache.tile([128, KT], f32r, tag=f"xh{cjt}_{dh}", name=f"xh{cjt}_{dh}")
                nc.gpsimd.dma_start(out=t[:], in_=xv[dh * 128:(dh + 1) * 128, cjt * KT:(cjt + 1) * KT])
                cache_t[(cjt, dh)] = t
        nkt_total = KLEN // KT
        for ckt in range(nkt_total - NCACHE_KT, nkt_total):
            for chh in range(2):
                cache_t[(ckt, chh)] = cache.tile(
                    [128, KT], f32r, tag=f"xc{ckt}_{chh}", name=f"xc{ckt}_{chh}"
                )

        # ---------------- Phase 1: Gram matrix ----------------
        with (
            tc.tile_pool(name="p1in", bufs=2) as p1in,
            tc.tile_pool(name="p1t", bufs=14) as p1t,
            tc.tile_pool(name="p1ps", bufs=4, space="PSUM") as p1ps,
            tc.tile_pool(name="gacc", bufs=1, space="PSUM") as gacc,
            tc.tile_pool(name="gsb", bufs=1) as gsb,
        ):
            # acc0 = [Ghh(ch0, :) | Ghl(ch0, :)]  (one group, own bank).
            # acc1 = [Ghh(ch1, ch1) | Ghl(ch1, :)] (384 wide): Ghh's (ch1,ch0)
            # quadrant is skipped (symmetry; reconstructed by transpose in ph2).
            # acc1 holds TWO groups in one bank: only the hi-group's pair-0 MM
            # uses start=True (clears the whole bank); the lo-group always uses
            # start=False and relies on that clear + PE program order.
            acc = [gacc.tile([128, 512], f32, name="acc0"),
                   gacc.tile([128, 384], f32, name="acc1")]

            nkt = KLEN // KT
            nsub = KT // 128
            npair_total = KLEN // 256
            pending = []  # software-pipeline: MMs lag the split by two pairs

            def emit_mms(xt2, pair_i):
                for k in range(2):
                    koff = k * 256
                    first = pair_i == 0 and k == 0
                    last = pair_i == npair_total - 1 and k == 1
                    nc.tensor.matmul(
                        acc[0][:],
                        xt2[:, 0, koff: koff + 128],
                        xt2[:, :, koff: koff + 256],
                        start=first, stop=last,
                    )
                    lh1 = xt2[:, 0, koff + 128: koff + 256]
                    nc.tensor.matmul(
                        acc[1][:, 0:128], lh1,
                        xt2[:, 0, koff + 128: koff + 256],
                        start=first, stop=last,
                    )
                    nc.tensor.matmul(
                        acc[1][:, 128:384], lh1,
                        xt2[:, 1, koff: koff + 256],
                        start=False, stop=last, skip_group_check=True,
                    )

            for kt in range(nkt):
                xf = []
                for chh in range(2):
                    t = p1in.tile([128, KT], f32, tag=f"xf{chh}", name=f"xf{chh}")
                    nc.sync.dma_start(out=t[:], in_=xv[chh * 128:(chh + 1) * 128, kt * KT:(kt + 1) * KT])
                    xf.append(t)
                    if (kt, chh) in cache_t:
                        nc.gpsimd.tensor_copy(out=cache_t[(kt, chh)][:], in_=t[:])
                for j2 in range(nsub // 2):
                    pair_i = kt * (nsub // 2) + j2
                    pst = p1ps.tile([128, 512], f32, tag="pst", name="pst")
                    for k in range(2):
                        js = slice((2 * j2 + k) * 128, (2 * j2 + k + 1) * 128)
                        nc.tensor.transpose(pst[:, k * 256: k * 256 + 128], xf[0][:, js], ident_f32[:])
                        nc.tensor.transpose(pst[:, k * 256 + 128: (k + 1) * 256], xf[1][:, js], ident_f32[:])
                    # xt2[:, 0, :] = [XhT(k0) | XhT(k1)], xt2[:, 1, :] = [XlT(k0) | XlT(k1)]
                    xt2 = p1t.tile([128, 2, 512], bf16, tag="xt", name="xt2")
                    nc.scalar.copy(out=xt2[:, 0, :], in_=pst[:])
                    nc.vector.tensor_sub(xt2[:, 1, :], pst[:], xt2[:, 0, :])
                    pending.append((xt2, pair_i))
                    if len(pending) > 6:
                        emit_mms(*pending.pop(0))
            for p in pending:
                emit_mms(*p)

            # ---------------- Phase 2: softmax + B ----------------
            ga0 = gsb.tile([128, 512], f32, name="ga0")
            nc.scalar.copy(out=ga0[:], in_=acc[0][:])
            ga1 = gsb.tile([128, 384], f32, name="ga1")
            nc.vector.tensor_copy(out=ga1[:], in_=acc[1][:])
            ga = [ga0, ga1]
            GHL_OFF = [256, 128]  # Ghl(dh, :) column offset within ga[dh]

            g_half = []
            # ch0 rows: Ghh(ch0,:) + Ghl(ch0,:) + GhlT(ch0,:)
            pt0 = p1ps.tile([128, C], f32, tag="pst", name="pt0")
            for dh in range(2):
                nc.tensor.transpose(
                    pt0[:, dh * 128:(dh + 1) * 128],
                    ga[dh][:, GHL_OFF[dh]: GHL_OFF[dh] + 128],
                    ident_f32[:],
                )
            g0 = gsb.tile([128, C], f32, name="g0")
            nc.vector.tensor_add(g0[:], ga0[:, 0:256], ga0[:, 256:512])
            nc.vector.tensor_add(g0[:], g0[:], pt0[:])
            g_half.append(g0)
            # ch1 rows: Ghh(ch1,ch0) reconstructed as T(Ghh(ch0,ch1))
            pt1 = p1ps.tile([128, 512], f32, tag="pst", name="pt1")
            nc.tensor.transpose(pt1[:, 0:128], ga0[:, 128:256], ident_f32[:])
            for dh in range(2):
                nc.tensor.transpose(
                    pt1[:, 128 + dh * 128: 128 + (dh + 1) * 128],
                    ga[dh][:, GHL_OFF[dh] + 128: GHL_OFF[dh] + 256],
                    ident_f32[:],
                )
            g1 = gsb.tile([128, C], f32, name="g1")
            nc.vector.tensor_add(g1[:, 0:128], pt1[:, 0:128], ga1[:, 128:256])
            nc.vector.tensor_add(g1[:, 0:128], g1[:, 0:128], pt1[:, 128:256])
            nc.vector.tensor_add(g1[:, 128:256], ga1[:, 0:128], ga1[:, 256:384])
            nc.vector.tensor_add(g1[:, 128:256], g1[:, 128:256], pt1[:, 256:384])
            g_half.append(g1)
            if debug:
                for chh in range(2):
                    nc.sync.dma_start(out=dbg["g_dbg"][chh * 128:(chh + 1) * 128, :], in_=g_half[chh][:])

            attn = []
            for chh in range(2):
                mn = gsb.tile([128, 1], f32, tag=f"mn{chh}", name=f"mn{chh}")
                nc.vector.tensor_reduce(mn[:], g_half[chh][:], axis=mybir.AxisListType.X, op=mybir.AluOpType.min)
                s = gsb.tile([128, C], f32, tag=f"s{chh}", name=f"s{chh}")
                ssum = gsb.tile([128, 1], f32, tag=f"ss{chh}", name=f"ss{chh}")
                nc.scalar.activation(
                    out=s[:], in_=g_half[chh][:],
                    func=mybir.ActivationFunctionType.Exp,
                    bias=mn[:], scale=-1.0, accum_out=ssum[:],
                )
                rinv = gsb.tile([128, 1], f32, tag=f"ri{chh}", name=f"ri{chh}")
                nc.vector.reciprocal(rinv[:], ssum[:])
                gm = gsb.tile([128, 1], f32, tag=f"gm{chh}", name=f"gm{chh}")
                nc.vector.tensor_mul(gm[:], rinv[:], g_sb[:])
                at = gsb.tile([128, C], f32, tag=f"at{chh}", name=f"at{chh}")
                nc.vector.tensor_scalar_mul(out=at[:], in0=s[:], scalar1=gm[:])
                attn.append(at)

            for dh in range(2):
                pb = p1ps.tile([128, C], f32, tag="pst", name="pb")
                for chh in range(2):
                    nc.tensor.transpose(
                        pb[:, chh * 128:(chh + 1) * 128],
                        attn[chh][:, dh * 128:(dh + 1) * 128],
                        ident_f32[:],
                    )
                nc.vector.tensor_add(b_t[dh][:], pb[:], eye[dh][:])
                if debug:
                    nc.sync.dma_start(out=dbg["b_dbg"][dh * 128:(dh + 1) * 128, :], in_=b_t[dh][:].bitcast(f32))

        # ---------------- Phase 3: out = B.T @ X (f32r) ----------------
        with (
            tc.tile_pool(name="p3in", bufs=2) as p3in,
            tc.tile_pool(name="p3out", bufs=3) as p3out,
            tc.tile_pool(name="p3ps", bufs=8, space="PSUM") as p3ps,
        ):
            njt = KLEN // JT
            jt_order = [jt for jt in range(njt) if (jt, 0) in cache_t] + \
                       [jt for jt in range(njt) if (jt, 0) not in cache_t]
            for jt in jt_order:
                if (jt * JT // KT, 0) in cache_t and JT == KT:
                    xr = [cache_t[(jt, dh)] for dh in range(2)]
                else:
                    xr = []
                    for dh in range(2):
                        t = p3in.tile([128, JT], f32r, tag=f"xr{dh}", name=f"xr{dh}")
                        nc.gpsimd.dma_start(out=t[:], in_=xv[dh * 128:(dh + 1) * 128, jt * JT:(jt + 1) * JT])
                        xr.append(t)
                for jp in range(JT // 1024):
                    for chh in range(2):
                        ot = p3out.tile([128, 1024], f32, tag=f"ot{chh}", name=f"ot{chh}")
                        po = [p3ps.tile([128, 512], f32, tag="po", name=f"po{_i}") for _i in range(2)]
                        for dh in range(2):
                            for jj in range(2):
                                col = slice((2 * jp + jj) * 512, (2 * jp + jj + 1) * 512)
                                nc.tensor.matmul(
                                    po[jj][:],
                                    b_t[dh][:, chh * 128:(chh + 1) * 128],
                                    xr[dh][:, col],
                                    start=(dh == 0), stop=(dh == 1),
                                )
                        for jj in range(2):
                            eng = nc.scalar.copy if jj == 0 else nc.vector.tensor_copy
                            eng(out=ot[:, jj * 512:(jj + 1) * 512], in_=po[jj][:])
                        nc.sync.dma_start(
                            out=ov[chh * 128:(chh + 1) * 128, jt * JT + jp * 1024: jt * JT + (jp + 1) * 1024],
                            in_=ot[:],
                        )

    nc.finalize()
    return nc


def _get_nc(n_seg: int):
    if n_seg not in _nc_cache:
        _nc_cache[n_seg] = _build(n_seg)
    return _nc_cache[n_seg]


def kernel(feats, gamma, _trace=False, _n_seg=N_SEG):
    from concourse.bass_utils import run_bass_kernel_spmd

    feats = np.asarray(feats, dtype=np.float32)
    gamma = np.asarray(gamma, dtype=np.float32)
    assert feats.shape == (BATCHES * _n_seg, C), feats.shape

    nc = _get_nc(_n_seg)
    xs = feats.reshape(BATCHES, _n_seg, C)
    in_maps = [
        {"x": np.ascontiguousarray(xs[i]), "gamma": gamma} for i in range(BATCHES)
    ]
    if _trace:
        try:
            from antenv.axon_hooks import get_axon_ntff_profile_hook  # noqa: F401
        except ImportError:
            _trace = False
    res = run_bass_kernel_spmd(nc, in_maps, core_ids=list(range(BATCHES)), trace=_trace)
    out = np.concatenate([r["out"] for r in res.results], axis=0)
    if _trace:
        kernel.last_results = res
    return out.astype(np.float32)



# revision 13
# speedup vs baseline: 1.6311x; 1.6311x over previous
"""TRN2 Bass kernel for nn_CAM_Module (channel attention over packed point-cloud scenes).

Math per segment (n rows, C=256 channels), with X = segment viewed as [C, n]
(a pure reshape of the row-major [n, C] buffer):
    G    = X @ X.T                      # [C, C] Gram over the flat axis
    attn = softmax(rowmax(G) - G)       # == exp(rowmin(G) - G) / rowsum (shift cancels)
    out  = gamma * (attn @ X) + X       # viewed back as [n, C]

Sharding: 8 segments -> 8 NeuronCores, fully local per core.

Implementation per core (all matmuls in f32r: full-rate PE at >=256-wide
output, f32-exact accumulate):
  Phase 1: stream X in [128, 2, KT] f32r chunks; PE-transpose 128-col
           subchunks (f32r identity), full-G f32r matmuls (2x256-wide per
           subchunk) accumulating into one PSUM bank; Pool casts the first
           NCACHE chunks to a persistent bf16 SBUF cache for phase 3.
  Phase 2: softmax straight off PSUM (DVE min-reduce, ACT exp+accum-sum,
           DVE reciprocal), fold gamma, PE-transpose to B^T = g*attn^T + I.
  Phase 3: out = B^T @ X with moving operand from the bf16 cache (no HBM
           read) or f32r re-read for uncached chunks; PSUM drained by
           ACT/DVE into bf16 tiles; output stored in bf16 (host upcasts).
           Loads ride SP/HWDGE, stores Pool/SWDGE so neither SEQ blocks
           the other.
"""

import numpy as np

BATCHES = 8
C = 256
N_SEG = 65536  # rows per segment

_nc_cache = {}


def _build(n_seg: int):
    """Emit the Bass program for one core (one segment of n_seg rows)."""
    from contextlib import ExitStack

    import concourse.bass as bass
    import concourse.tile as tile
    from concourse import bacc, mybir
    from concourse.masks import make_identity

    f32 = mybir.dt.float32
    f32r = mybir.dt.float32r
    bf16 = mybir.dt.bfloat16

    KLEN = n_seg  # flat axis length per channel row
    KT = 1024  # k-cols per chunk
    assert KLEN % KT == 0
    NCHUNK = KLEN // KT
    NCACHE = min(38, NCHUNK)  # chunks cached in SBUF as bf16 for phase 3
    MM_LAG = 3  # pairs of lookahead between transpose and matmul emission

    nc = bacc.Bacc("TRN2", target_bir_lowering=False, debug=False, num_devices=8)

    x = nc.dram_tensor("x", [n_seg, C], f32, kind="ExternalInput").ap()
    gamma = nc.dram_tensor("gamma", [1], f32, kind="ExternalInput").ap()
    out = nc.dram_tensor("out", [n_seg, C], bf16, kind="ExternalOutput").ap()

    # [128, 2, KLEN] views: partition = channel-within-half, dim1 = ch half.
    xw = x.rearrange("(h c r) ch -> c h (r ch)", h=2, c=128)
    ow = out.rearrange("(h c r) ch -> c h (r ch)", h=2, c=128)

    with tile.TileContext(nc) as tc, ExitStack() as ctx:
        const = ctx.enter_context(tc.tile_pool(name="const", bufs=1))
        cache = ctx.enter_context(tc.tile_pool(name="xcache", bufs=1))

        ident = const.tile([128, 128], f32)
        make_identity(nc, ident[:])
        # f32r-typed identity for phase-1 transposes (the BIR verifier wants
        # fp32r matmult operands produced by fp32r-writing instructions)
        identr = const.tile([128, 128], f32r, tag="identr", name="identr")
        nc.gpsimd.tensor_copy(out=identr[:], in_=ident[:])
        ident_r = identr[:]

        # I_dh[p, c] = 1.0 iff c == p + 128*dh   (residual identity, [d, c])
        eye = []
        for dh in range(2):
            t = const.tile([128, C], f32, tag=f"eye{dh}", name=f"eye{dh}")
            nc.gpsimd.memset(t[:], 0.0)
            nc.gpsimd.affine_select(
                out=t[:],
                in_=t[:],
                compare_op=mybir.AluOpType.not_equal,
                fill=1.0,
                base=128 * dh,
                pattern=[[-1, C]],
                channel_multiplier=1,
            )
            eye.append(t)

        g_sb = const.tile([128, 1], f32)
        g_bcast = bass.AP(tensor=gamma.tensor, offset=gamma.offset, ap=[[0, 128], [1, 1]])
        nc.gpsimd.dma_start(out=g_sb[:], in_=g_bcast)

        # B^T tiles (gamma*attn^T + I), [d-half, c-full]; filled in phase 2.
        # f32r copy pairs with f32r re-read chunks, bf16 with the bf16 cache
        # (the BIR verifier requires matching dtypes when either is f32/f32r).
        b_t = [const.tile([128, C], f32r, tag=f"bt{dh}", name=f"bt{dh}") for dh in range(2)]
        b16 = [const.tile([128, C], bf16, tag=f"bh{dh}", name=f"bh{dh}") for dh in range(2)]

        # bf16 SBUF cache of the first NCACHE chunks (filled during phase 1)
        cache_t = {
            kt: cache.tile([128, 2, KT], bf16, tag=f"xc{kt}", name=f"xc{kt}")
            for kt in range(NCACHE)
        }

        # ---------------- Phase 1: Gram matrix ----------------
        with (
            tc.tile_pool(name="p1in", bufs=3) as p1in,
            tc.tile_pool(name="p1t", bufs=6) as p1t,
            tc.tile_pool(name="p1ps", bufs=4, space="PSUM") as p1ps,
            tc.tile_pool(name="gacc", bufs=1, space="PSUM") as gacc,
            tc.tile_pool(name="gsb", bufs=1) as gsb,
        ):
            # Full G in one PSUM bank: rows ch-half 0 at [:, 0:256], half 1
            # at [:, 256:512]. One start=True clears the bank (pending-zero
            # covers the zero region); everything else accumulates.
            acc = gacc.tile([128, 512], f32, name="acc")

            npair = KLEN // 256
            pending = []  # software pipeline: MMs lag the transposes

            def emit_mms(xt, pair_i):
                for k in range(2):
                    first = pair_i == 0 and k == 0
                    last = pair_i == npair - 1 and k == 1
                    for chh in range(2):
                        nc.tensor.matmul(
                            acc[:, chh * 256:(chh + 1) * 256],
                            xt[:, k * 256 + chh * 128: k * 256 + chh * 128 + 128],
                            xt[:, k * 256:(k + 1) * 256],
                            start=first and chh == 0,
                            stop=last,
                            skip_group_check=not (first and chh == 0),
                        )

            for kt in range(NCHUNK):
                xf = p1in.tile([128, 2, KT], f32r, tag="xf", name="xf")
                nc.sync.dma_start(out=xf[:], in_=xw[:, :, kt * KT:(kt + 1) * KT].bitcast(f32r))
                if kt in cache_t:
                    nc.gpsimd.tensor_copy(out=cache_t[kt][:], in_=xf[:].bitcast(f32))
                for j2 in range(KT // 256):
                    pair_i = kt * (KT // 256) + j2
                    pst = p1ps.tile([128, 512], f32r, tag="pst", name="pst")
                    for k in range(2):
                        js = slice((2 * j2 + k) * 128, (2 * j2 + k + 1) * 128)
                        for h in range(2):
                            nc.tensor.transpose(
                                pst[:, k * 256 + h * 128: k * 256 + (h + 1) * 128],
                                xf[:, h, js],
                                ident_r,
                            )
                    xt = p1t.tile([128, 512], f32r, tag="xt", name="xt")
                    eng = nc.scalar.copy if j2 % 2 == 0 else nc.vector.tensor_copy
                    eng(out=xt[:], in_=pst[:])
                    pending.append((xt, pair_i))
                    if len(pending) > MM_LAG:
                        emit_mms(*pending.pop(0))
            for p in pending:
                emit_mms(*p)

            # ---------------- Phase 2: softmax + B^T ----------------
            attn = []
            for chh in range(2):
                gs = acc[:, chh * 256:(chh + 1) * 256]
                mn = gsb.tile([128, 1], f32, tag=f"mn{chh}", name=f"mn{chh}")
                nc.vector.tensor_reduce(mn[:], gs, axis=mybir.AxisListType.X, op=mybir.AluOpType.min)
                s = gsb.tile([128, C], f32, tag=f"s{chh}", name=f"s{chh}")
                ssum = gsb.tile([128, 1], f32, tag=f"ss{chh}", name=f"ss{chh}")
                nc.scalar.activation(
                    out=s[:], in_=gs,
                    func=mybir.ActivationFunctionType.Exp,
                    bias=mn[:], scale=-1.0, accum_out=ssum[:],
                )
                rinv = gsb.tile([128, 1], f32, tag=f"ri{chh}", name=f"ri{chh}")
                nc.vector.reciprocal(rinv[:], ssum[:])
                gm = gsb.tile([128, 1], f32, tag=f"gm{chh}", name=f"gm{chh}")
                nc.vector.tensor_mul(gm[:], rinv[:], g_sb[:])
                at = gsb.tile([128, C], f32, tag=f"at{chh}", name=f"at{chh}")
                nc.vector.tensor_scalar_mul(out=at[:], in0=s[:], scalar1=gm[:])
                attn.append(at)

            for dh in range(2):
                pb = p1ps.tile([128, C], f32, tag="pst", name="pb")
                for chh in range(2):
                    nc.tensor.transpose(
                        pb[:, chh * 128:(chh + 1) * 128],
                        attn[chh][:, dh * 128:(dh + 1) * 128],
                        ident[:],
                    )
                nc.vector.tensor_add(b_t[dh][:], pb[:], eye[dh][:])
                nc.gpsimd.tensor_copy(out=b16[dh][:], in_=b_t[dh][:].bitcast(f32))

        # ---------------- Phase 3: out = B^T @ X ----------------
        with (
            tc.tile_pool(name="p3in", bufs=3) as p3in,
            tc.tile_pool(name="p3out", bufs=4) as p3out,
            tc.tile_pool(name="p3ps", bufs=4, space="PSUM") as p3ps,
        ):
            uncached = [kt for kt in range(NCHUNK) if kt not in cache_t]
            order = list(cache_t.keys()) + uncached
            xr_pending = {}
            load_iter = iter(uncached)

            def issue_load():
                kt = next(load_iter, None)
                if kt is not None:
                    t = p3in.tile([128, 2, KT], f32r, tag="xr", name="xr")
                    nc.sync.dma_start(out=t[:], in_=xw[:, :, kt * KT:(kt + 1) * KT].bitcast(f32r))
                    xr_pending[kt] = t

            for _ in range(3):
                issue_load()

            for jt in order:
                if jt in cache_t:
                    mov, b_use = cache_t[jt], b16
                else:
                    mov, b_use = xr_pending.pop(jt), b_t
                    issue_load()
                for chh in range(2):
                    ot = p3out.tile([128, KT], bf16, tag=f"ot{chh}", name=f"ot{chh}")
                    for jp in range(KT // 512):
                        po = p3ps.tile([128, 512], f32, tag="po", name="po")
                        for dh in range(2):
                            nc.tensor.matmul(
                                po[:],
                                b_use[dh][:, chh * 128:(chh + 1) * 128],
                                mov[:, dh, jp * 512:(jp + 1) * 512],
                                start=dh == 0, stop=dh == 1,
                            )
                        eng = nc.scalar.copy if jp % 2 == 0 else nc.vector.tensor_copy
                        eng(out=ot[:, jp * 512:(jp + 1) * 512], in_=po[:])
                    nc.gpsimd.dma_start(
                        out=ow[:, chh, jt * KT:(jt + 1) * KT],
                        in_=ot[:],
                    )

    nc.finalize()
    return nc


def _get_nc(n_seg: int):
    if n_seg not in _nc_cache:
        _nc_cache[n_seg] = _build(n_seg)
    return _nc_cache[n_seg]


def kernel(feats, gamma, _trace=False, _n_seg=N_SEG):
    from concourse.bass_utils import run_bass_kernel_spmd

    feats = np.asarray(feats, dtype=np.float32)
    gamma = np.asarray(gamma, dtype=np.float32)
    assert feats.shape == (BATCHES * _n_seg, C), feats.shape

    nc = _get_nc(_n_seg)
    xs = feats.reshape(BATCHES, _n_seg, C)
    in_maps = [
        {"x": np.ascontiguousarray(xs[i]), "gamma": gamma} for i in range(BATCHES)
    ]
    if _trace:
        try:
            from antenv.axon_hooks import get_axon_ntff_profile_hook  # noqa: F401
        except ImportError:
            _trace = False
    res = run_bass_kernel_spmd(nc, in_maps, core_ids=list(range(BATCHES)), trace=_trace)
    out = np.concatenate([np.asarray(r["out"]) for r in res.results], axis=0)
    if _trace:
        kernel.last_results = res
    return out.astype(np.float32)


# revision 52
# speedup vs baseline: 1.7338x; 1.0630x over previous
"""TRN2 Bass kernel for nn_CAM_Module (channel attention over packed point-cloud scenes).

Math per segment (n rows, C=256 channels), with X = segment viewed as [C, n]
(a pure reshape of the row-major [n, C] buffer):
    G    = X @ X.T                      # [C, C] Gram over the flat axis
    attn = softmax(rowmax(G) - G)       # == exp(rowmin(G) - G) / rowsum (shift cancels)
    out  = gamma * (attn @ X) + X       # viewed back as [n, C]

Sharding: 8 segments -> 8 NeuronCores, fully local per core.

Implementation per core (all matmuls in f32r: full-rate PE at >=256-wide
output, f32-exact accumulate):
  Phase 1: stream X in [128, 2, KT] f32r chunks; PE-transpose 128-col
           subchunks (f32r identity), full-G f32r matmuls (2x256-wide per
           subchunk) accumulating into one PSUM bank; Pool casts the first
           NCACHE chunks to a persistent bf16 SBUF cache for phase 3.
  Phase 2: softmax straight off PSUM (DVE min-reduce, ACT exp+accum-sum,
           DVE reciprocal), fold gamma, PE-transpose to B^T = g*attn^T + I.
  Phase 3: out = B^T @ X with moving operand from the bf16 cache (no HBM
           read) or f32r re-read for uncached chunks; PSUM drained by
           ACT/DVE into bf16 tiles; output stored in bf16 (host upcasts).
           Loads ride SP/HWDGE, stores Pool/SWDGE so neither SEQ blocks
           the other.
"""

import numpy as np

BATCHES = 8
C = 256
N_SEG = 65536  # rows per segment

_nc_cache = {}


def _build(n_seg: int):
    """Emit the Bass program for one core (one segment of n_seg rows)."""
    from contextlib import ExitStack

    import concourse.bass as bass
    import concourse.tile as tile
    from concourse import bacc, mybir
    from concourse.masks import make_identity

    f32 = mybir.dt.float32
    f32r = mybir.dt.float32r
    bf16 = mybir.dt.bfloat16

    KLEN = n_seg  # flat axis length per channel row
    KT = 1024  # k-cols per chunk
    assert KLEN % KT == 0
    NCHUNK = KLEN // KT
    NCACHE = min(38, NCHUNK)  # chunks cached in SBUF as bf16 for phase 3
    MM_LAG = 5  # pairs of lookahead between transpose and matmul emission

    nc = bacc.Bacc("TRN2", target_bir_lowering=False, debug=False, num_devices=8)

    x = nc.dram_tensor("x", [n_seg, C], f32, kind="ExternalInput").ap()
    gamma = nc.dram_tensor("gamma", [1], f32, kind="ExternalInput").ap()
    out = nc.dram_tensor("out", [n_seg, C], bf16, kind="ExternalOutput").ap()

    # [128, 2, KLEN] views: partition = channel-within-half, dim1 = ch half.
    xw = x.rearrange("(h c r) ch -> c h (r ch)", h=2, c=128)
    ow = out.rearrange("(h c r) ch -> c h (r ch)", h=2, c=128)

    with tile.TileContext(nc) as tc, ExitStack() as ctx:
        const = ctx.enter_context(tc.tile_pool(name="const", bufs=1))
        cache = ctx.enter_context(tc.tile_pool(name="xcache", bufs=1))

        ident = const.tile([128, 128], f32)
        make_identity(nc, ident[:])
        # f32r-typed identity for phase-1 transposes (the BIR verifier wants
        # fp32r matmult operands produced by fp32r-writing instructions)
        identr = const.tile([128, 128], f32r, tag="identr", name="identr")
        nc.gpsimd.tensor_copy(out=identr[:], in_=ident[:])
        ident_r = identr[:]

        # I_dh[p, c] = 1.0 iff c == p + 128*dh   (residual identity, [d, c])
        eye = []
        for dh in range(2):
            t = const.tile([128, C], f32, tag=f"eye{dh}", name=f"eye{dh}")
            nc.gpsimd.memset(t[:], 0.0)
            nc.gpsimd.affine_select(
                out=t[:],
                in_=t[:],
                compare_op=mybir.AluOpType.not_equal,
                fill=1.0,
                base=128 * dh,
                pattern=[[-1, C]],
                channel_multiplier=1,
            )
            eye.append(t)

        g_sb = const.tile([128, 1], f32)
        g_bcast = bass.AP(tensor=gamma.tensor, offset=gamma.offset, ap=[[0, 128], [1, 1]])
        nc.gpsimd.dma_start(out=g_sb[:], in_=g_bcast)

        # B^T tiles (gamma*attn^T + I), [d-half, c-full]; filled in phase 2.
        # f32r copy pairs with f32r re-read chunks, bf16 with the bf16 cache
        # (the BIR verifier requires matching dtypes when either is f32/f32r).
        b_t = [const.tile([128, C], f32r, tag=f"bt{dh}", name=f"bt{dh}") for dh in range(2)]
        b16 = [const.tile([128, C], bf16, tag=f"bh{dh}", name=f"bh{dh}") for dh in range(2)]

        # bf16 SBUF cache of the first NCACHE chunks (filled during phase 1)
        cache_t = {
            kt: cache.tile([128, 2, KT], bf16, tag=f"xc{kt}", name=f"xc{kt}")
            for kt in range(NCACHE)
        }

        # ---------------- Phase 1: Gram matrix ----------------
        # gacc/gsb/p1ps persist through phase 2; p1in/p1t close right after
        # the streaming loop so phase-3 loads (which reuse their SBUF range)
        # only wait on phase-1 readers, overlapping the softmax with DMA.
        if True:
            gsb = ctx.enter_context(tc.tile_pool(name="gsb", bufs=1))
            ps_stack = ExitStack()  # PSUM pools, closed after phase 2
            gacc = ps_stack.enter_context(tc.tile_pool(name="gacc", bufs=1, space="PSUM"))
            p1ps = ps_stack.enter_context(tc.tile_pool(name="p1ps", bufs=4, space="PSUM"))
            # Full G in one PSUM bank: rows ch-half 0 at [:, 0:256], half 1
            # at [:, 256:512]. One start=True clears the bank (pending-zero
            # covers the zero region); everything else accumulates.
            acc = gacc.tile([128, 512], f32, name="acc")

            npair = KLEN // 256
            pending = []  # software pipeline: MMs lag the transposes

            def emit_mms(xt, pair_i):
                for k in range(2):
                    first = pair_i == 0 and k == 0
                    last = pair_i == npair - 1 and k == 1
                    for chh in range(2):
                        nc.tensor.matmul(
                            acc[:, chh * 256:(chh + 1) * 256],
                            xt[:, k * 256 + chh * 128: k * 256 + chh * 128 + 128],
                            xt[:, k * 256:(k + 1) * 256],
                            start=first and chh == 0,
                            stop=last,
                            skip_group_check=not (first and chh == 0),
                        )

            with (
                tc.tile_pool(name="p1in", bufs=3) as p1in,
                tc.tile_pool(name="p1in0", bufs=1) as p1in0,
                tc.tile_pool(name="p1t", bufs=6) as p1t,
            ):
              for kt in range(NCHUNK):
                if kt == 0:
                    # split the first chunk's load so the PE pipeline starts
                    # after half a transfer instead of a full one
                    half = []
                    for hh in range(2):
                        t = p1in0.tile([128, 2, KT // 2], f32r, tag=f"xf0{hh}", name=f"xf0{hh}")
                        nc.sync.dma_start(
                            out=t[:],
                            in_=xw[:, :, hh * (KT // 2):(hh + 1) * (KT // 2)].bitcast(f32r))
                        half.append(t)
                    if kt in cache_t:
                        for hh in range(2):
                            nc.gpsimd.tensor_copy(
                                out=cache_t[kt][:, :, hh * (KT // 2):(hh + 1) * (KT // 2)],
                                in_=half[hh][:].bitcast(f32))
                else:
                    xf = p1in.tile([128, 2, KT], f32r, tag="xf", name="xf")
                    nc.sync.dma_start(out=xf[:], in_=xw[:, :, kt * KT:(kt + 1) * KT].bitcast(f32r))
                    if kt in cache_t:
                        nc.gpsimd.tensor_copy(out=cache_t[kt][:], in_=xf[:].bitcast(f32))
                for j2 in range(KT // 256):
                    pair_i = kt * (KT // 256) + j2
                    pst = p1ps.tile([128, 512], f32r, tag="pst", name="pst")
                    for k in range(2):
                        col = (2 * j2 + k) * 128
                        if kt == 0:
                            src = half[col // (KT // 2)]
                            js = slice(col % (KT // 2), col % (KT // 2) + 128)
                        else:
                            src = xf
                            js = slice(col, col + 128)
                        for h in range(2):
                            nc.tensor.transpose(
                                pst[:, k * 256 + h * 128: k * 256 + (h + 1) * 128],
                                src[:, h, js],
                                ident_r,
                            )
                    xt = p1t.tile([128, 512], f32r, tag="xt", name="xt")
                    eng = nc.scalar.copy if j2 % 2 == 0 else nc.vector.tensor_copy
                    eng(out=xt[:], in_=pst[:])
                    pending.append((xt, pair_i))
                    if len(pending) > MM_LAG:
                        emit_mms(*pending.pop(0))
              for p in pending:
                emit_mms(*p)

            # Phase-3 input pool + first prefetches, emitted BEFORE phase 2:
            # the loads alias p1in/p1t SBUF (freed at the loop end), so they
            # overlap the softmax below instead of waiting behind it.
            p3in = ctx.enter_context(tc.tile_pool(name="p3in", bufs=4))
            uncached = [kt for kt in range(NCHUNK) if kt not in cache_t]
            cached = sorted(cache_t)
            c_main, c_tail = cached[:-3], cached[-3:]
            # merge by fractional position so re-read DMA spreads evenly
            n = len(c_main) + len(uncached)
            keyed = [((i + 0.5) * n / len(c_main), kt) for i, kt in enumerate(c_main)] + \
                    [((i + 0.5) * n / len(uncached), kt) for i, kt in enumerate(uncached)]
            order = [kt for _, kt in sorted(keyed)] + c_tail
            xr_pending = {}
            load_iter = iter(uncached)

            def issue_load():
                kt = next(load_iter, None)
                if kt is not None:
                    t = p3in.tile([128, 2, KT], f32r, tag="xr", name="xr")
                    nc.sync.dma_start(out=t[:], in_=xw[:, :, kt * KT:(kt + 1) * KT].bitcast(f32r))
                    xr_pending[kt] = t

            for _ in range(4):
                issue_load()

            # ---------------- Phase 2: softmax + B^T ----------------
            mk = lambda tag, sh: gsb.tile(sh, f32, tag=tag, name=tag)
            mn = [mk(f"mn{c}", [128, 1]) for c in range(2)]
            sx = [mk(f"s{c}", [128, C]) for c in range(2)]
            ssum = [mk(f"ss{c}", [128, 1]) for c in range(2)]
            rinv = [mk(f"ri{c}", [128, 1]) for c in range(2)]
            gm = [mk(f"gm{c}", [128, 1]) for c in range(2)]
            attn = [mk(f"at{c}", [128, C]) for c in range(2)]
            for chh in range(2):
                nc.vector.tensor_reduce(mn[chh][:], acc[:, chh * 256:(chh + 1) * 256],
                                        axis=mybir.AxisListType.X, op=mybir.AluOpType.min)
            for chh in range(2):
                nc.scalar.activation(
                    out=sx[chh][:], in_=acc[:, chh * 256:(chh + 1) * 256],
                    func=mybir.ActivationFunctionType.Exp,
                    bias=mn[chh][:], scale=-1.0, accum_out=ssum[chh][:],
                )
            for chh in range(2):
                nc.vector.reciprocal(rinv[chh][:], ssum[chh][:])
            for chh in range(2):
                ve = nc.vector if chh == 0 else nc.gpsimd
                ve.tensor_mul(gm[chh][:], rinv[chh][:], g_sb[:])
                ve.tensor_scalar_mul(out=attn[chh][:], in0=sx[chh][:], scalar1=gm[chh][:])

            for dh in range(2):
                pb = p1ps.tile([128, C], f32, tag="pst", name="pb")
                for chh in range(2):
                    nc.tensor.transpose(
                        pb[:, chh * 128:(chh + 1) * 128],
                        attn[chh][:, dh * 128:(dh + 1) * 128],
                        ident[:],
                    )
                nc.vector.tensor_add(b_t[dh][:], pb[:], eye[dh][:])
                eb = nc.gpsimd if dh == 0 else nc.vector
                eb.tensor_copy(out=b16[dh][:], in_=b_t[dh][:].bitcast(f32))

        ps_stack.close()  # free phase-1/2 PSUM banks for phase 3

        # ---------------- Phase 3: out = B^T @ X ----------------
        with (
            tc.tile_pool(name="p3out", bufs=3) as p3out,
            tc.tile_pool(name="p3ps", bufs=8, space="PSUM") as p3ps,
        ):
            for jt in order:
                if jt in cache_t:
                    mov, b_use = cache_t[jt], b16
                else:
                    mov, b_use = xr_pending.pop(jt), b_t
                    issue_load()
                for chh in range(2):
                    ot = p3out.tile([128, KT], bf16, tag=f"ot{chh}", name=f"ot{chh}")
                    for jp in range(KT // 512):
                        po = p3ps.tile([128, 512], f32, tag="po", name="po")
                        for dh in range(2):
                            nc.tensor.matmul(
                                po[:],
                                b_use[dh][:, chh * 128:(chh + 1) * 128],
                                mov[:, dh, jp * 512:(jp + 1) * 512],
                                start=dh == 0, stop=dh == 1,
                            )
                        eng = nc.scalar.copy if jp % 2 == 0 else nc.vector.tensor_copy
                        eng(out=ot[:, jp * 512:(jp + 1) * 512], in_=po[:])
                    st = nc.gpsimd if chh == 0 else nc.scalar
                    st.dma_start(
                        out=ow[:, chh, jt * KT:(jt + 1) * KT],
                        in_=ot[:],
                    )

    nc.finalize()
    return nc


def _get_nc(n_seg: int):
    if n_seg not in _nc_cache:
        _nc_cache[n_seg] = _build(n_seg)
    return _nc_cache[n_seg]


def kernel(feats, gamma, _trace=False, _n_seg=N_SEG):
    from concourse.bass_utils import run_bass_kernel_spmd

    feats = np.asarray(feats, dtype=np.float32)
    gamma = np.asarray(gamma, dtype=np.float32)
    assert feats.shape == (BATCHES * _n_seg, C), feats.shape

    nc = _get_nc(_n_seg)
    xs = feats.reshape(BATCHES, _n_seg, C)
    in_maps = [
        {"x": np.ascontiguousarray(xs[i]), "gamma": gamma} for i in range(BATCHES)
    ]
    if _trace:
        try:
            from antenv.axon_hooks import get_axon_ntff_profile_hook  # noqa: F401
        except ImportError:
            _trace = False
    res = run_bass_kernel_spmd(nc, in_maps, core_ids=list(range(BATCHES)), trace=_trace)
    out = np.concatenate([np.asarray(r["out"]) for r in res.results], axis=0)
    if _trace:
        kernel.last_results = res
    return out.astype(np.float32)
